# revision 5
# baseline (speedup 1.0000x reference)
"""Bahdanau additive attention on 8 Trainium2 NeuronCores.

  score_t = V^T tanh(W1 value_t + W2 query);  out = softmax(score) @ value

Sharding: data-parallel over batch (16 batches -> 2 per core), weights
replicated.  Inside each core the kernel is memory-bound: value (16 MiB
fp32 per core) is read from HBM exactly once via a casting DMA
(fp32 -> bf16 in flight), transposed on-chip with the DMA xbar
(SBUF->SBUF), and consumed by three PE matmul streams (keys, score,
context) plus one ACT tanh stream.
"""

import functools
import os
import sys

import numpy as np

if "/opt/trn_rl_repo" not in sys.path:
    sys.path.insert(0, "/opt/trn_rl_repo")

B, T, D, U = 16, 8192, 256, 256
NCORES = 8
BPC = B // NCORES          # batches per core
P = 128                    # SBUF partitions
ST = 1024                  # t per supertile
NST = T // ST              # supertiles per batch
CH = 512                   # score/keys chunk width (PSUM bank = 512 fp32)
NCH = ST // CH             # chunks per supertile


@functools.lru_cache(maxsize=1)
def _build():
    from contextlib import ExitStack

    import concourse.bass as bass
    import concourse.tile as tile
    from concourse import bacc, mybir
    from concourse.masks import make_identity

    f32 = mybir.dt.float32
    bf16 = mybir.dt.bfloat16
    Act = mybir.ActivationFunctionType

    nc = bacc.Bacc("TRN2", target_bir_lowering=False, debug=False)

    q = nc.dram_tensor("query", [BPC, D], f32, kind="ExternalInput").ap()
    val = nc.dram_tensor("value", [BPC, T, D], f32, kind="ExternalInput").ap()
    w1 = nc.dram_tensor("W1", [D, U], f32, kind="ExternalInput").ap()
    w2 = nc.dram_tensor("W2", [D, U], f32, kind="ExternalInput").ap()
    vv = nc.dram_tensor("V", [U, 1], f32, kind="ExternalInput").ap()
    out = nc.dram_tensor("out", [BPC, D], f32, kind="ExternalOutput").ap()

    with tile.TileContext(nc) as tc, ExitStack() as ctx:
        consts = ctx.enter_context(tc.tile_pool(name="consts", bufs=1))
        vpool = ctx.enter_context(tc.tile_pool(name="vbf", bufs=1))
        vtpool = ctx.enter_context(tc.tile_pool(name="vt", bufs=3))
        thpool = ctx.enter_context(tc.tile_pool(name="th", bufs=4))
        scpool = ctx.enter_context(tc.tile_pool(name="scsb", bufs=1))
        small = ctx.enter_context(tc.tile_pool(name="small", bufs=1))
        psk = ctx.enter_context(tc.tile_pool(name="psk", bufs=3, space="PSUM"))
        pssc = ctx.enter_context(tc.tile_pool(name="pssc", bufs=2, space="PSUM"))
        psmisc = ctx.enter_context(tc.tile_pool(name="psmisc", bufs=2, space="PSUM"))
        psctx = ctx.enter_context(tc.tile_pool(name="psctx", bufs=1, space="PSUM"))

        # ---- constants / weights -------------------------------------
        ident = consts.tile([64, 64], f32)
        make_identity(nc, ident)
        ones = consts.tile([P, 1], f32)
        nc.vector.memset(ones, 1.0)

        w1b = consts.tile([P, 2, U], bf16)
        nc.gpsimd.dma_start(out=w1b, in_=w1.rearrange("(kb p) u -> p kb u", p=P))
        w2b = consts.tile([P, 2, U], f32)
        nc.sync.dma_start(out=w2b, in_=w2.rearrange("(kb p) u -> p kb u", p=P))
        vsb = consts.tile([P, 2, 1], bf16)
        nc.gpsimd.dma_start(out=vsb, in_=vv.rearrange("(ub p) o -> p ub o", p=P))

        # hidden = query @ W2, computed as hidden^T [u, b] so it can feed the
        # tanh as a per-partition bias.
        q_sb = consts.tile([BPC, D], f32)
        nc.sync.dma_start(out=q_sb, in_=q)
        qt = consts.tile([P, 2, BPC], f32)
        for kb in range(2):
            psq = psmisc.tile([P, BPC], f32, tag="misc")
            nc.tensor.transpose(
                out=psq, in_=q_sb[:, P * kb:P * (kb + 1)], identity=ident[0:BPC, 0:BPC]
            )
            nc.vector.tensor_copy(out=qt[:, kb, :], in_=psq)
        hid = []
        for u in range(2):
            psh = psmisc.tile([P, BPC], f32, tag="misc")
            for kb in range(2):
                nc.tensor.matmul(
                    psh,
                    lhsT=w2b[:, kb, P * u:P * (u + 1)],
                    rhs=qt[:, kb, :],
                    start=(kb == 0),
                    stop=(kb == 1),
                )
            h = consts.tile([P, BPC], f32, tag=f"hid{u}")
            nc.vector.tensor_copy(out=h, in_=psh)
            hid.append(h)

        ctx_sb = consts.tile([P, D], f32)

        # ---- main streaming loop -------------------------------------
        vb = [[None] * NST for _ in range(BPC)]
        sc_sb = [None] * BPC
        for b in range(BPC):
            sc_sb[b] = scpool.tile(
                [P, NST, CH], f32, tag=f"scsb{b}", name=f"scsb{b}"
            )
            for j in range(NST):
                VB = vpool.tile([P, ST // P, D], bf16, tag=f"vb_{b}_{j}")
                vb[b][j] = VB
                nc.gpsimd.dma_start(
                    out=VB,
                    in_=val[b, ST * j:ST * (j + 1), :].rearrange(
                        "(f p) d -> p f d", p=P
                    ),
                )
                psSC = pssc.tile([P, CH], f32)
                for c in range(NCH):
                    VT = vtpool.tile([P, 2, CH], bf16)
                    for m in range(CH // P):
                        f = (CH // P) * c + m
                        for h in range(2):
                            nc.sync.dma_start(
                                out=VT[:, h, P * m:P * (m + 1)],
                                in_=VB[:, f, P * h:P * (h + 1)],
                                transpose=True,
                            )
                    ths = []
                    for u in range(2):
                        psK = psk.tile([P, CH], f32)
                        for kb in range(2):
                            nc.tensor.matmul(
                                psK,
                                lhsT=w1b[:, kb, P * u:P * (u + 1)],
                                rhs=VT[:, kb, :],
                                start=(kb == 0),
                                stop=(kb == 1),
                            )
                        th = thpool.tile([P, CH], bf16)
                        nc.scalar.activation(
                            out=th,
                            in_=psK,
                            func=Act.Tanh,
                            bias=hid[u][:, b:b + 1],
                            scale=1.0,
                        )
                        ths.append(th)
                    row = 64 * b + 32 * c
                    for u in range(2):
                        nc.tensor.matmul(
                            psSC[row:row + 1, :],
                            lhsT=vsb[:, u, :],
                            rhs=ths[u],
                            start=(u == 0),
                            stop=(u == 1),
                            tile_position=(0, row),
                        )
                nc.vector.tensor_copy(out=sc_sb[b][:, j, :], in_=psSC)

            # ---- per-batch tail: softmax + context -------------------
            # Gather this batch's scores into S64 [64, 128]; row r = 32c+4j+k
            # holds t-chunk chi(r) = 8j + 4c + k (see context loop below).
            S64 = small.tile([64, P], f32, tag=f"s64_{b}")
            for c in range(NCH):
                row = 64 * b + 32 * c
                for j in range(NST):
                    nc.gpsimd.dma_start(
                        out=S64[32 * c + 4 * j:32 * c + 4 * j + 4, :],
                        in_=sc_sb[b][row:row + 1, j, :].rearrange(
                            "o (k f) -> o k f", k=4
                        ),
                    )
            psTS = psmisc.tile([P, 64], f32, tag="misc")
            nc.tensor.transpose(out=psTS, in_=S64, identity=ident)
            e128 = small.tile([P, 64], bf16, tag=f"e_{b}")
            nc.scalar.activation(out=e128, in_=psTS, func=Act.Exp, scale=1.0)
            pb = small.tile([P, 1], f32, tag=f"pb_{b}")
            nc.vector.reduce_sum(out=pb, in_=e128, axis=mybir.AxisListType.X)
            psS = psmisc.tile([P, 1], f32, tag="misc")
            nc.tensor.matmul(
                psS[32 * b:32 * b + 1, :], lhsT=ones, rhs=pb, start=True, stop=True
            )
            invS = small.tile([P, 1], f32, tag=f"invs_{b}")
            nc.vector.reciprocal(
                out=invS[32 * b:32 * b + 1, :], in_=psS[32 * b:32 * b + 1, :]
            )

            psC = psctx.tile([P, D], f32, tag="psctx")
            for r in range(64):
                c, j, k = r // 32, (r % 32) // 4, r % 4
                f = (CH // P) * c + k
                nc.tensor.matmul(
                    psC[32 * b:32 * b + 1, :],
                    lhsT=e128[:, r:r + 1],
                    rhs=vb[b][j][:, f, :],
                    start=(r == 0),
                    stop=(r == 63),
                )
            nc.vector.tensor_scalar_mul(
                out=ctx_sb[32 * b:32 * b + 1, :],
                in0=psC[32 * b:32 * b + 1, :],
                scalar1=invS[32 * b:32 * b + 1, :],
            )
            nc.sync.dma_start(out=out[b:b + 1, :], in_=ctx_sb[32 * b:32 * b + 1, :])

    nc.finalize()
    return nc


def _run(inputs, trace=False):
    from concourse import bass_utils

    nc = _build()
    in_maps = [
        {
            "query": np.ascontiguousarray(inputs["query"][BPC * i:BPC * (i + 1)]),
            "value": np.ascontiguousarray(inputs["value"][BPC * i:BPC * (i + 1)]),
            "W1": np.asarray(inputs["W1"]),
            "W2": np.asarray(inputs["W2"]),
            "V": np.asarray(inputs["V"]),
        }
        for i in range(NCORES)
    ]
    res = bass_utils.run_bass_kernel_spmd(
        nc, in_maps, core_ids=list(range(NCORES)), trace=trace
    )
    outp = np.concatenate([r["out"] for r in res.results], axis=0)
    return outp.astype(np.float32), res


def kernel(**inputs) -> np.ndarray:
    outp, _ = _run(inputs, trace=False)
    return outp


# revision 7
# speedup vs baseline: 2.1747x; 2.1747x over previous
"""Bahdanau additive attention on 8 Trainium2 NeuronCores.

  score_t = V^T tanh(W1 value_t + W2 query);  out = softmax(score) @ value

Sharding: data-parallel over batch (16 batches -> 2 per core), weights
replicated.  Inside each core the kernel is memory-bound: value (16 MiB
fp32 per core) is read from HBM exactly once via a casting DMA
(fp32 -> bf16 in flight), transposed on-chip with the DMA xbar
(SBUF->SBUF), and consumed by three PE matmul streams (keys, score,
context) plus one ACT tanh stream.
"""

import functools
import os
import sys

import numpy as np

if "/opt/trn_rl_repo" not in sys.path:
    sys.path.insert(0, "/opt/trn_rl_repo")

B, T, D, U = 16, 8192, 256, 256
NCORES = 8
BPC = B // NCORES          # batches per core
P = 128                    # SBUF partitions
ST = 1024                  # t per supertile
NST = T // ST              # supertiles per batch
CH = 512                   # score/keys chunk width (PSUM bank = 512 fp32)
NCH = ST // CH             # chunks per supertile


@functools.lru_cache(maxsize=1)
def _build():
    from contextlib import ExitStack

    import concourse.bass as bass
    import concourse.tile as tile
    from concourse import bacc, mybir
    from concourse.masks import make_identity

    f32 = mybir.dt.float32
    bf16 = mybir.dt.bfloat16
    Act = mybir.ActivationFunctionType

    nc = bacc.Bacc("TRN2", target_bir_lowering=False, debug=False)

    q = nc.dram_tensor("query", [BPC, D], f32, kind="ExternalInput").ap()
    val = nc.dram_tensor("value", [BPC, T, D], f32, kind="ExternalInput").ap()
    w1 = nc.dram_tensor("W1", [D, U], f32, kind="ExternalInput").ap()
    w2 = nc.dram_tensor("W2", [D, U], f32, kind="ExternalInput").ap()
    vv = nc.dram_tensor("V", [U, 1], f32, kind="ExternalInput").ap()
    out = nc.dram_tensor("out", [BPC, D], f32, kind="ExternalOutput").ap()

    with tile.TileContext(nc) as tc, ExitStack() as ctx:
        consts = ctx.enter_context(tc.tile_pool(name="consts", bufs=1))
        vpool = ctx.enter_context(tc.tile_pool(name="vbf", bufs=1))
        vtpool = ctx.enter_context(tc.tile_pool(name="vt", bufs=3))
        thpool = ctx.enter_context(tc.tile_pool(name="th", bufs=4))
        scpool = ctx.enter_context(tc.tile_pool(name="scsb", bufs=1))
        small = ctx.enter_context(tc.tile_pool(name="small", bufs=1))
        psk = ctx.enter_context(tc.tile_pool(name="psk", bufs=3, space="PSUM"))
        pssc = ctx.enter_context(tc.tile_pool(name="pssc", bufs=2, space="PSUM"))
        psmisc = ctx.enter_context(tc.tile_pool(name="psmisc", bufs=2, space="PSUM"))
        psctx = ctx.enter_context(tc.tile_pool(name="psctx", bufs=1, space="PSUM"))

        # ---- constants / weights -------------------------------------
        ident = consts.tile([64, 64], f32)
        make_identity(nc, ident)
        ones = consts.tile([P, 1], f32)
        nc.vector.memset(ones, 1.0)

        w1b = consts.tile([P, 2, U], bf16)
        nc.gpsimd.dma_start(out=w1b, in_=w1.rearrange("(kb p) u -> p kb u", p=P))
        w2b = consts.tile([P, 2, U], f32)
        nc.sync.dma_start(out=w2b, in_=w2.rearrange("(kb p) u -> p kb u", p=P))
        vsb = consts.tile([P, 2, 1], bf16)
        nc.gpsimd.dma_start(out=vsb, in_=vv.rearrange("(ub p) o -> p ub o", p=P))

        # hidden = query @ W2, computed as hidden^T [u, b] so it can feed the
        # tanh as a per-partition bias.
        q_sb = consts.tile([BPC, D], f32)
        nc.sync.dma_start(out=q_sb, in_=q)
        qt = consts.tile([P, 2, BPC], f32)
        for kb in range(2):
            psq = psmisc.tile([P, BPC], f32, tag="misc")
            nc.tensor.transpose(
                out=psq, in_=q_sb[:, P * kb:P * (kb + 1)], identity=ident[0:BPC, 0:BPC]
            )
            nc.vector.tensor_copy(out=qt[:, kb, :], in_=psq)
        hid = []
        for u in range(2):
            psh = psmisc.tile([P, BPC], f32, tag="misc")
            for kb in range(2):
                nc.tensor.matmul(
                    psh,
                    lhsT=w2b[:, kb, P * u:P * (u + 1)],
                    rhs=qt[:, kb, :],
                    start=(kb == 0),
                    stop=(kb == 1),
                )
            h = consts.tile([P, BPC], f32, tag=f"hid{u}")
            nc.vector.tensor_copy(out=h, in_=psh)
            hid.append(h)

        ctx_sb = consts.tile([P, D], f32)

        # ---- main streaming loop -------------------------------------
        vb = [[None] * NST for _ in range(BPC)]
        sc_sb = [None] * BPC
        for b in range(BPC):
            sc_sb[b] = scpool.tile(
                [P, NST, CH], f32, tag=f"scsb{b}", name=f"scsb{b}"
            )
            for j in range(NST):
                # VB[p, h, f, d'] = value[t = 128f + p, d = 128h + d']
                VB = vpool.tile([P, 2, ST // P, P], bf16, tag=f"vb_{b}_{j}")
                vb[b][j] = VB
                for h in range(2):
                    nc.gpsimd.dma_start(
                        out=VB[:, h],
                        in_=val[
                            b, ST * j:ST * (j + 1), P * h:P * (h + 1)
                        ].rearrange("(f p) d -> p f d", p=P),
                    )
                # One batched xbar transpose per d-half:
                # VT[h][d', f, t'] = VB[t', h, f, d']  (value^T)
                VT = [None, None]
                for h in range(2):
                    vt_h = vtpool.tile(
                        [P, ST // P, P], bf16, tag="vt", name=f"vt_{b}_{j}_{h}"
                    )
                    nc.sync.dma_start(out=vt_h, in_=VB[:, h], transpose=True)
                    VT[h] = vt_h
                psSC = pssc.tile([P, CH], f32)
                for c in range(NCH):
                    ths = []
                    for u in range(2):
                        psK = psk.tile([P, CH], f32)
                        for kb in range(2):
                            nc.tensor.matmul(
                                psK,
                                lhsT=w1b[:, kb, P * u:P * (u + 1)],
                                rhs=VT[kb][
                                    :, (CH // P) * c:(CH // P) * (c + 1), :
                                ].rearrange("p f t -> p (f t)"),
                                start=(kb == 0),
                                stop=(kb == 1),
                            )
                        th = thpool.tile([P, CH], bf16)
                        nc.scalar.activation(
                            out=th,
                            in_=psK,
                            func=Act.Tanh,
                            bias=hid[u][:, b:b + 1],
                            scale=1.0,
                        )
                        ths.append(th)
                    row = 64 * b + 32 * c
                    for u in range(2):
                        nc.tensor.matmul(
                            psSC[row:row + 1, :],
                            lhsT=vsb[:, u, :],
                            rhs=ths[u],
                            start=(u == 0),
                            stop=(u == 1),
                            tile_position=(0, row),
                        )
                nc.vector.tensor_copy(out=sc_sb[b][:, j, :], in_=psSC)

            # ---- per-batch tail: softmax + context -------------------
            # Gather this batch's scores into S64 [64, 128]; row r = 32c+4j+k
            # holds t-chunk chi(r) = 8j + 4c + k (see context loop below).
            S64 = small.tile([64, P], f32, tag=f"s64_{b}")
            for c in range(NCH):
                row = 64 * b + 32 * c
                for j in range(NST):
                    nc.gpsimd.dma_start(
                        out=S64[32 * c + 4 * j:32 * c + 4 * j + 4, :],
                        in_=sc_sb[b][row:row + 1, j, :].rearrange(
                            "o (k f) -> o k f", k=4
                        ),
                    )
            psTS = psmisc.tile([P, 64], f32, tag="misc")
            nc.tensor.transpose(out=psTS, in_=S64, identity=ident)
            e128 = small.tile([P, 64], bf16, tag=f"e_{b}")
            nc.scalar.activation(out=e128, in_=psTS, func=Act.Exp, scale=1.0)
            pb = small.tile([P, 1], f32, tag=f"pb_{b}")
            nc.vector.reduce_sum(out=pb, in_=e128, axis=mybir.AxisListType.X)
            psS = psmisc.tile([P, 1], f32, tag="misc")
            nc.tensor.matmul(
                psS[32 * b:32 * b + 1, :], lhsT=ones, rhs=pb, start=True, stop=True
            )
            invS = small.tile([P, 1], f32, tag=f"invs_{b}")
            nc.vector.reciprocal(
                out=invS[32 * b:32 * b + 1, :], in_=psS[32 * b:32 * b + 1, :]
            )

            psC = psctx.tile([P, D], f32, tag="psctx")
            for r in range(64):
                c, j, k = r // 32, (r % 32) // 4, r % 4
                f = (CH // P) * c + k
                nc.tensor.matmul(
                    psC[32 * b:32 * b + 1, :],
                    lhsT=e128[:, r:r + 1],
                    rhs=vb[b][j][:, :, f, :],
                    start=(r == 0),
                    stop=(r == 63),
                )
            nc.vector.tensor_scalar_mul(
                out=ctx_sb[32 * b:32 * b + 1, :],
                in0=psC[32 * b:32 * b + 1, :],
                scalar1=invS[32 * b:32 * b + 1, :],
            )
            nc.sync.dma_start(out=out[b:b + 1, :], in_=ctx_sb[32 * b:32 * b + 1, :])

    nc.finalize()
    return nc


def _run(inputs, trace=False):
    from concourse import bass_utils

    nc = _build()
    in_maps = [
        {
            "query": np.ascontiguousarray(inputs["query"][BPC * i:BPC * (i + 1)]),
            "value": np.ascontiguousarray(inputs["value"][BPC * i:BPC * (i + 1)]),
            "W1": np.asarray(inputs["W1"]),
            "W2": np.asarray(inputs["W2"]),
            "V": np.asarray(inputs["V"]),
        }
        for i in range(NCORES)
    ]
    res = bass_utils.run_bass_kernel_spmd(
        nc, in_maps, core_ids=list(range(NCORES)), trace=trace
    )
    outp = np.concatenate([r["out"] for r in res.results], axis=0)
    return outp.astype(np.float32), res


def kernel(**inputs) -> np.ndarray:
    outp, _ = _run(inputs, trace=False)
    return outp


# revision 9
# speedup vs baseline: 3.1146x; 1.4322x over previous
"""Bahdanau additive attention on 8 Trainium2 NeuronCores.

  score_t = V^T tanh(W1 value_t + W2 query);  out = softmax(score) @ value

Sharding: data-parallel over batch (16 batches -> 2 per core), weights
replicated.  Inside each core the kernel is memory-bound: value (16 MiB
fp32 per core) is read from HBM exactly once via a casting DMA
(fp32 -> bf16 in flight), transposed on-chip with the DMA xbar
(SBUF->SBUF), and consumed by three PE matmul streams (keys, score,
context) plus one ACT tanh stream.
"""

import functools
import os
import sys

import numpy as np

if "/opt/trn_rl_repo" not in sys.path:
    sys.path.insert(0, "/opt/trn_rl_repo")

B, T, D, U = 16, 8192, 256, 256
NCORES = 8
BPC = B // NCORES          # batches per core
P = 128                    # SBUF partitions
ST = 1024                  # t per supertile
NST = T // ST              # supertiles per batch
CH = 512                   # score/keys chunk width (PSUM bank = 512 fp32)
NCH = ST // CH             # chunks per supertile


@functools.lru_cache(maxsize=1)
def _build():
    from contextlib import ExitStack

    import concourse.bass as bass
    import concourse.tile as tile
    from concourse import bacc, mybir
    from concourse.masks import make_identity

    f32 = mybir.dt.float32
    bf16 = mybir.dt.bfloat16
    Act = mybir.ActivationFunctionType

    nc = bacc.Bacc("TRN2", target_bir_lowering=False, debug=False)

    q = nc.dram_tensor("query", [BPC, D], f32, kind="ExternalInput").ap()
    val = nc.dram_tensor("value", [BPC, T, D], f32, kind="ExternalInput").ap()
    w1 = nc.dram_tensor("W1", [D, U], f32, kind="ExternalInput").ap()
    w2 = nc.dram_tensor("W2", [D, U], f32, kind="ExternalInput").ap()
    vv = nc.dram_tensor("V", [U, 1], f32, kind="ExternalInput").ap()
    out = nc.dram_tensor("out", [BPC, D], f32, kind="ExternalOutput").ap()

    with tile.TileContext(nc) as tc, ExitStack() as ctx:
        consts = ctx.enter_context(tc.tile_pool(name="consts", bufs=1))
        vpool = ctx.enter_context(tc.tile_pool(name="vbf", bufs=1))
        v32pool = ctx.enter_context(tc.tile_pool(name="v32", bufs=3))
        vtpool = ctx.enter_context(tc.tile_pool(name="vt", bufs=3))
        thpool = ctx.enter_context(tc.tile_pool(name="th", bufs=4))
        scpool = ctx.enter_context(tc.tile_pool(name="scsb", bufs=1))
        small = ctx.enter_context(tc.tile_pool(name="small", bufs=1))
        psk = ctx.enter_context(tc.tile_pool(name="psk", bufs=3, space="PSUM"))
        pssc = ctx.enter_context(tc.tile_pool(name="pssc", bufs=2, space="PSUM"))
        psmisc = ctx.enter_context(tc.tile_pool(name="psmisc", bufs=2, space="PSUM"))
        psctx = ctx.enter_context(tc.tile_pool(name="psctx", bufs=1, space="PSUM"))

        # ---- constants / weights -------------------------------------
        ident = consts.tile([64, 64], f32)
        make_identity(nc, ident)
        ones = consts.tile([P, 1], f32)
        nc.vector.memset(ones, 1.0)

        w1b = consts.tile([P, 2, U], bf16)
        nc.gpsimd.dma_start(out=w1b, in_=w1.rearrange("(kb p) u -> p kb u", p=P))
        w2b = consts.tile([P, 2, U], f32)
        nc.sync.dma_start(out=w2b, in_=w2.rearrange("(kb p) u -> p kb u", p=P))
        vsb = consts.tile([P, 2, 1], bf16)
        nc.gpsimd.dma_start(out=vsb, in_=vv.rearrange("(ub p) o -> p ub o", p=P))

        # hidden = query @ W2, computed as hidden^T [u, b] so it can feed the
        # tanh as a per-partition bias.
        q_sb = consts.tile([BPC, D], f32)
        nc.sync.dma_start(out=q_sb, in_=q)
        qt = consts.tile([P, 2, BPC], f32)
        for kb in range(2):
            psq = psmisc.tile([P, BPC], f32, tag="misc")
            nc.tensor.transpose(
                out=psq, in_=q_sb[:, P * kb:P * (kb + 1)], identity=ident[0:BPC, 0:BPC]
            )
            nc.vector.tensor_copy(out=qt[:, kb, :], in_=psq)
        hid = []
        for u in range(2):
            psh = psmisc.tile([P, BPC], f32, tag="misc")
            for kb in range(2):
                nc.tensor.matmul(
                    psh,
                    lhsT=w2b[:, kb, P * u:P * (u + 1)],
                    rhs=qt[:, kb, :],
                    start=(kb == 0),
                    stop=(kb == 1),
                )
            h = consts.tile([P, BPC], f32, tag=f"hid{u}")
            nc.vector.tensor_copy(out=h, in_=psh)
            hid.append(h)

        ctx_sb = consts.tile([P, D], f32)

        # ---- main streaming loop -------------------------------------
        vb = [[None] * NST for _ in range(BPC)]
        sc_sb = [None] * BPC
        for b in range(BPC):
            sc_sb[b] = scpool.tile(
                [P, NST, CH], f32, tag=f"scsb{b}", name=f"scsb{b}"
            )
            for j in range(NST):
                # Plain fp32 load (HWDGE, 1 KiB lines): VB32[p, f, d]
                VB32 = v32pool.tile([P, ST // P, D], f32, tag="vb32", name="vb32")
                nc.sync.dma_start(
                    out=VB32,
                    in_=val[b, ST * j:ST * (j + 1), :].rearrange(
                        "(f p) d -> p f d", p=P
                    ),
                )
                # On-chip cast to bf16, reshaping so each d-half is a
                # contiguous 128-run: VB[p, h, f, d'] = value[128f+p, 128h+d']
                VB = vpool.tile([P, 2, ST // P, P], bf16, tag=f"vb_{b}_{j}")
                vb[b][j] = VB
                nc.vector.tensor_copy(
                    out=VB,
                    in_=VB32.rearrange("p f (h d) -> p h f d", h=2),
                )
                # One batched xbar transpose per supertile:
                # VT[d', (h f), t'] = VB[t', h, f, d']  (value^T blocks)
                VT = vtpool.tile([P, 2, ST // P, P], bf16, tag="vt", name="vt")
                nc.sync.dma_start(
                    out=VT.rearrange("p h f t -> p (h f) t"),
                    in_=VB.rearrange("p h f t -> p (h f t)"),
                    transpose=True,
                )
                psSC = pssc.tile([P, CH], f32)
                for c in range(NCH):
                    ths = []
                    for u in range(2):
                        psK = psk.tile([P, CH], f32)
                        for kb in range(2):
                            nc.tensor.matmul(
                                psK,
                                lhsT=w1b[:, kb, P * u:P * (u + 1)],
                                rhs=VT[
                                    :, kb, (CH // P) * c:(CH // P) * (c + 1), :
                                ].rearrange("p f t -> p (f t)"),
                                start=(kb == 0),
                                stop=(kb == 1),
                            )
                        th = thpool.tile([P, CH], bf16)
                        nc.scalar.activation(
                            out=th,
                            in_=psK,
                            func=Act.Tanh,
                            bias=hid[u][:, b:b + 1],
                            scale=1.0,
                        )
                        ths.append(th)
                    row = 64 * b + 32 * c
                    for u in range(2):
                        nc.tensor.matmul(
                            psSC[row:row + 1, :],
                            lhsT=vsb[:, u, :],
                            rhs=ths[u],
                            start=(u == 0),
                            stop=(u == 1),
                            tile_position=(0, row),
                        )
                nc.vector.tensor_copy(out=sc_sb[b][:, j, :], in_=psSC)

            # ---- per-batch tail: softmax + context -------------------
            # Gather this batch's scores into S64 [64, 128]; row r = 32c+4j+k
            # holds t-chunk chi(r) = 8j + 4c + k (see context loop below).
            S64 = small.tile([64, P], f32, tag=f"s64_{b}")
            for c in range(NCH):
                row = 64 * b + 32 * c
                for j in range(NST):
                    nc.gpsimd.dma_start(
                        out=S64[32 * c + 4 * j:32 * c + 4 * j + 4, :],
                        in_=sc_sb[b][row:row + 1, j, :].rearrange(
                            "o (k f) -> o k f", k=4
                        ),
                    )
            psTS = psmisc.tile([P, 64], f32, tag="misc")
            nc.tensor.transpose(out=psTS, in_=S64, identity=ident)
            e128 = small.tile([P, 64], bf16, tag=f"e_{b}")
            nc.scalar.activation(out=e128, in_=psTS, func=Act.Exp, scale=1.0)
            pb = small.tile([P, 1], f32, tag=f"pb_{b}")
            nc.vector.reduce_sum(out=pb, in_=e128, axis=mybir.AxisListType.X)
            psS = psmisc.tile([P, 1], f32, tag="misc")
            nc.tensor.matmul(
                psS[32 * b:32 * b + 1, :], lhsT=ones, rhs=pb, start=True, stop=True
            )
            invS = small.tile([P, 1], f32, tag=f"invs_{b}")
            nc.vector.reciprocal(
                out=invS[32 * b:32 * b + 1, :], in_=psS[32 * b:32 * b + 1, :]
            )

            psC = psctx.tile([P, D], f32, tag="psctx")
            for r in range(64):
                c, j, k = r // 32, (r % 32) // 4, r % 4
                f = (CH // P) * c + k
                nc.tensor.matmul(
                    psC[32 * b:32 * b + 1, :],
                    lhsT=e128[:, r:r + 1],
                    rhs=vb[b][j][:, :, f, :],
                    start=(r == 0),
                    stop=(r == 63),
                )
            nc.vector.tensor_scalar_mul(
                out=ctx_sb[32 * b:32 * b + 1, :],
                in0=psC[32 * b:32 * b + 1, :],
                scalar1=invS[32 * b:32 * b + 1, :],
            )
            nc.sync.dma_start(out=out[b:b + 1, :], in_=ctx_sb[32 * b:32 * b + 1, :])

    nc.finalize()
    return nc


def _run(inputs, trace=False):
    from concourse import bass_utils

    nc = _build()
    in_maps = [
        {
            "query": np.ascontiguousarray(inputs["query"][BPC * i:BPC * (i + 1)]),
            "value": np.ascontiguousarray(inputs["value"][BPC * i:BPC * (i + 1)]),
            "W1": np.asarray(inputs["W1"]),
            "W2": np.asarray(inputs["W2"]),
            "V": np.asarray(inputs["V"]),
        }
        for i in range(NCORES)
    ]
    res = bass_utils.run_bass_kernel_spmd(
        nc, in_maps, core_ids=list(range(NCORES)), trace=trace
    )
    outp = np.concatenate([r["out"] for r in res.results], axis=0)
    return outp.astype(np.float32), res


def kernel(**inputs) -> np.ndarray:
    outp, _ = _run(inputs, trace=False)
    return outp
